# revision 48
# baseline (speedup 1.0000x reference)
# Distributed sparse-attention kernel for Trainium2 (8 NeuronCores).
#
# Sharding: core c = (batch b = c//2, head-group g = c%2 of 8 heads).
# Each core computes, for its (b, g):
#   q  = meancenter(x) @ Wc          (LN rstd cancels under l2norm; gamma and
#                                     mean-centering folded into Wc on host)
#   kv = [prefix; x] @ Wkv.T         (MQA single head, replicated per batch)
#   qn = l2norm(q) ; kn = l2norm(k) * (8 * q_scale * k_scale)
#   logits[c_key, i_query] = kn.T qn  (transposed; causal triangle over x-cols
#                                     packed edge-to-edge in psum, 16-wide band
#                                     over prefix-cols)
#   logits += bias                   (additive ln-domain bias incl. causal /
#                                     key-mask kills, accumulated into psum by
#                                     identity matmuls on the PE)
#   P = exp(logits)                  (single activation, final P)
#   avT[d, i] = sum_j P[j, i] v[j, d], denom via an appended ones column of v
#   outT_partial = Wo_g.T @ (avT / denom)
# Host sums the two head-group partials per batch and transposes back.

import numpy as np

B, N, P, DIM, HEADS, DH = 4, 1024, 1024, 1024, 16, 64
HL = 8                 # heads per core
FL = HL * DH           # 512 local q features
J = P + N              # 2048 keys
WIND = 16              # prefix cond-window
BW = 144               # band tile width (128 cols + 16 window - 1, padded)
NEGB = -30000.0        # masked-logit bias (finite: 0 * -inf = NaN in matmuls)
CORES = list(range(8))

# packed psum column layout for the causal sims, per query-chunk qc:
# list of single-bank (512-wide) tiles; each tile is a list of
# (ct, pack_off, off_q, width); the Exp covers only valid columns.
PACK = {
    0: [[(0, 0, 0, 512)],
        [(1, 0, 128, 384), (3, 384, 384, 128)],
        [(2, 0, 256, 256)]],
    1: [[(0, 0, 0, 512)], [(1, 0, 0, 512)], [(2, 0, 0, 512)],
        [(3, 0, 0, 512)], [(4, 0, 0, 512)],
        [(5, 0, 128, 384), (7, 384, 384, 128)],
        [(6, 0, 256, 256)]],
}
USED = {0: [512, 512, 256], 1: [512] * 6 + [256]}
PTW = 3584             # per-(head, qc) packed width: max tiles (7) * 512


def _build_consts():
    """One packed [128, W] bf16 constant tensor; returns (array, offsets)."""
    cols = {}
    parts = []
    w = 0

    def add(name, arr):
        nonlocal w
        a = np.zeros((128, arr.shape[-1] if arr.ndim == 2 else
                      arr.shape[1] * arr.shape[2]), np.float32)
        if arr.ndim == 2:
            a[:arr.shape[0], :] = arr
        else:
            a[:arr.shape[0], :] = arr.reshape(arr.shape[0], -1)
        cols[name] = w
        parts.append(a)
        w += a.shape[1]

    # idup: v-transpose identity  [128, 64]
    add("idup", (np.arange(128)[:, None] % 64 == np.arange(64)[None, :])
        .astype(np.float32))
    # i128: full identity for psum bias-accumulate matmuls
    add("i128", np.eye(128, dtype=np.float32))
    # lband2: additive band mask, two 256-strided window copies [128, 512]
    lb = np.full((128, 512), NEGB, np.float32)
    r = np.arange(128)[:, None]
    t = np.arange(256)[None, :]
    win = ((t - r >= 0) & (t - r < WIND))
    lb[:, 0:256][win] = 0.0
    lb[:, 256:512][win] = 0.0
    add("lband2", lb)
    # indk[d, c, r] = (r == c): k ssq accumulate rows  [64, 4, 4]
    indk = np.zeros((64, 4, 4), np.float32)
    for c in range(4):
        indk[:, c, c] = 1.0
    add("indk", indk)
    # indq[d, c, r]: q ssq chunk c -> rows 2*(c%4), 2*(c%4)+1 of its half-batch
    indq = np.zeros((128, 8, 8), np.float32)
    for c in range(8):
        indq[0:64, c, 2 * (c % 4)] = 1.0
        indq[64:128, c, 2 * (c % 4) + 1] = 1.0
    add("indq", indq)
    # selk[k, c, p]: broadcast rsqK row c -> 64 partitions  [4, 4, 64]
    selk = np.zeros((4, 4, 64), np.float32)
    for c in range(4):
        selk[c, c, :] = 1.0
    add("selk", selk)
    # selq[k, c, p]: broadcast rsqQ half-batch rows  [8, 8, 128]
    selq = np.zeros((8, 8, 128), np.float32)
    for c in range(8):
        selq[2 * (c % 4), c, 0:64] = 1.0
        selq[2 * (c % 4) + 1, c, 64:128] = 1.0
    add("selq", selq)
    # selh[k, h, p]: broadcast denom-reciprocal row h  [8, 8, 64]
    selh = np.zeros((8, 8, 64), np.float32)
    for h in range(8):
        selh[h, h, :] = 1.0
    add("selh", selh)
    # selh1[p, h, j]: collect denom row -> psum row h (contract 1)  [128, 8, 8]
    selh1 = np.zeros((128, 8, 8), np.float32)
    for h in range(8):
        selh1[:, h, h] = 1.0
    add("selh1", selh1)

    return np.concatenate(parts, axis=1), cols


def _patch_tile_drain():
    """walrus in this image only encodes ~2 sem waits on a CTRL (Drain/Nop)
    instruction; Tile's exit drain attaches every outstanding sem wait to a
    single drain.  Split the waits across extra sync-engine nops."""
    import concourse.tile as tile_mod
    from concourse import mybir
    from concourse.vector_clock import ScopedClock

    if getattr(tile_mod.TileContext, "_drain_split_patch", False):
        return
    MAXW = 1

    _ENGS = {
        mybir.EngineType.PE, mybir.EngineType.Activation,
        mybir.EngineType.Pool, mybir.EngineType.DVE, mybir.EngineType.SP,
    }
    _LIMITS = {}
    _nsplit = [0]
    orig_add = tile_mod.TileContext._add_instruction

    def _add_instruction(self, inst):
        si = inst.sync_info
        lim = _LIMITS.get(inst.engine, 1)
        if (si is not None and si.on_wait and len(si.on_wait) > lim
                and inst.engine in _ENGS):
            waits = list(si.on_wait)
            keep = waits[:lim]
            rest = waits[lim:]
            inst.sync_info = mybir.SyncInfo(
                on_wait=keep, on_update=list(si.on_update or []))
            for i in range(0, len(rest), MAXW):
                _nsplit[0] += 1
                nop = mybir.InstNoOp(
                    name=f"{inst.name}-ws{_nsplit[0]}", ins=[], outs=[])
                nop.engine = inst.engine
                nop.sync_info = mybir.SyncInfo(
                    on_wait=rest[i:i + MAXW], on_update=[])
                orig_add(self, nop)
        orig_add(self, inst)

    tile_mod.TileContext._add_instruction = _add_instruction

    def _drain_and_barrier(self, tick_clock, wait_clock):
        drain_inst = self.nc.sync.drain()
        wait_clock.add_sem_waits(
            drain_inst.ins, ScopedClock({None: tick_clock.global_clock})
        )
        si = drain_inst.ins.sync_info
        waits = list(si.on_wait or []) if si is not None else []
        if len(waits) > MAXW:
            ups = list(si.on_update or []) if si is not None else []
            drain_inst.ins.sync_info = mybir.SyncInfo(on_wait=[], on_update=ups)
            for i in range(0, len(waits), MAXW):
                nop = self.nc.sync.nop(nofuse=True)
                nop.ins.sync_info = mybir.SyncInfo(
                    on_wait=waits[i:i + MAXW], on_update=[])
        self.nc.all_engine_barrier()
        assert self.sems is not None
        popped = self.nc._tile_sem_poison_stack.pop()
        assert popped is self._sem_poison
        self.nc.clear_and_free_semaphores(list(self.sems.allocated().values()))
        self.nc.all_engine_barrier()

    tile_mod.TileContext._drain_and_barrier = _drain_and_barrier
    tile_mod.TileContext._drain_split_patch = True


def _build_nc():
    import ml_dtypes
    import concourse.bass as bass
    import concourse.tile as tile
    from concourse import mybir

    _patch_tile_drain()

    f32 = mybir.dt.float32
    bf16 = mybir.dt.bfloat16
    bf = ml_dtypes.bfloat16

    nc = bass.Bass("TRN2", target_bir_lowering=False, debug=False)

    xT = nc.dram_tensor("xT", [128, 8, N], bf16, kind="ExternalInput").ap()
    ctxT = nc.dram_tensor("ctxT", [128, 8, P], bf16,
                          kind="ExternalInput").ap()
    wc = nc.dram_tensor("wc", [128, 8, FL], bf16, kind="ExternalInput").ap()
    wkv = nc.dram_tensor("wkv", [128, 8, 2 * DH], bf16,
                         kind="ExternalInput").ap()
    wo = nc.dram_tensor("wo", [128, 4, DIM], bf16, kind="ExternalInput").ap()
    # additive ln-domain bias, packed to the psum column layout
    biasP = nc.dram_tensor("biasP", [HL * 2, 128, PTW], bf16,
                           kind="ExternalInput").ap()
    sdk = nc.dram_tensor("sdk", [DH, 1], f32, kind="ExternalInput").ap()
    outT = nc.dram_tensor("outT", [DIM, N], bf16, kind="ExternalOutput").ap()

    cst_np, CO = _build_consts()
    cst_dram = nc.inline_tensor(cst_np.astype(bf), "cstp").ap()

    Exp = mybir.ActivationFunctionType.Exp
    Ln = mybir.ActivationFunctionType.Ln

    with tile.TileContext(nc) as tc, \
            tc.tile_pool(name="big", bufs=1) as big, \
            tc.tile_pool(name="cst", bufs=1) as cst, \
            tc.tile_pool(name="sq", bufs=2) as sqp, \
            tc.tile_pool(name="nrm", bufs=1) as nrm, \
            tc.tile_pool(name="ptx", bufs=4) as ptxp, \
            tc.tile_pool(name="btp", bufs=4) as btp, \
            tc.tile_pool(name="ptb", bufs=8) as ptbp, \
            tc.tile_pool(name="rec", bufs=2) as recp, \
            tc.tile_pool(name="osb", bufs=3) as osbp, \
            tc.tile_pool(name="pSim", bufs=4, space="PSUM") as pSim, \
            tc.tile_pool(name="pAv", bufs=2, space="PSUM") as pAv, \
            tc.tile_pool(name="pAux", bufs=2, space="PSUM") as pAux:

        # ---- phase A: finely-chunked loads so the kv/q matmuls trickle-start
        # as data lands; kv-path first ----
        ctxT_sb = big.tile([128, 8, P], bf16, tag="ctxT")
        nc.sync.dma_start(ctxT_sb[:, 0:1, :], ctxT[:, 0:1, :])
        nc.sync.dma_start(ctxT_sb[:, 1:2, :], ctxT[:, 1:2, :])
        for kt in range(2, 8, 2):
            nc.sync.dma_start(ctxT_sb[:, kt:kt + 2, :], ctxT[:, kt:kt + 2, :])
        wkv_sb = big.tile([128, 8, 2 * DH], bf16, tag="wkv")
        nc.gpsimd.dma_start(wkv_sb[:], wkv)
        cst_sb = cst.tile([128, cst_np.shape[1]], bf16, tag="cstp")
        nc.gpsimd.dma_start(cst_sb[:], cst_dram)
        wc_sb = big.tile([128, 8, FL], bf16, tag="wc")
        nc.gpsimd.dma_start(wc_sb[:], wc)
        xT_sb = big.tile([128, 8, N], bf16, tag="xT")
        for kt in range(0, 8, 2):
            eng = nc.gpsimd if kt < 4 else nc.sync
            eng.dma_start(xT_sb[:, kt:kt + 2, :], xT[:, kt:kt + 2, :])
        wo_sb = big.tile([128, 4, DIM], bf16, tag="wo")
        nc.gpsimd.dma_start(wo_sb[:], wo)
        sdk_sb = cst.tile([DH, 1], f32, tag="sdk")
        nc.gpsimd.dma_start(sdk_sb[:], sdk)
        eps_sb = cst.tile([128, 1], f32, tag="eps")
        nc.vector.memset(eps_sb[:], 1e-24)

        def cview(name, shape=None):
            o = CO[name]
            if shape is None:
                return cst_sb[:, o:o + 128]
            w = int(np.prod(shape[1:]))
            v = cst_sb[:, o:o + w]
            if len(shape) == 3:
                v = v.rearrange("p (a b) -> p a b", b=shape[2])
                if shape[0] < 128:
                    v = v[0:shape[0], :, :]
            elif shape[0] < 128:
                v = v[0:shape[0], :]
            return v

        idup_sb = cst_sb[:, CO["idup"]:CO["idup"] + 64]
        i128_sb = cview("i128")
        lband2_sb = cst_sb[:, CO["lband2"]:CO["lband2"] + 512]
        indk_sb = cview("indk", (64, 4, 4))
        indq_sb = cview("indq", (128, 8, 8))
        selk_sb = cview("selk", (4, 4, 64))
        selq_sb = cview("selq", (8, 8, 128))
        selh_sb = cview("selh", (8, 8, 64))
        selh1_sb = cview("selh1", (128, 8, 8))

        kvT_sb = big.tile([128, J], bf16, tag="kvT")      # [2d, j] raw kv
        kn_sb = big.tile([128, J], bf16, tag="kn")        # normalized k, dup'd
        va_sb = big.tile([128, 16, DH + 1], bf16, tag="va")  # v_aug, j-major
        qn_sb = big.tile([128, 4, N], bf16, tag="qn")     # normalized q
        att_sb = big.tile([128, 4, N], bf16, tag="att")   # avT/denom

        # ---- kv projection (fp8 DoubleRow) + k sumsq ----
        kssq = pAux.tile([128, 512], f32, tag="aux", name="kssq")

        Square = mybir.ActivationFunctionType.Square

        def emit_kv(jh):
            # two psum banks interleaved so accumulate fills/drains overlap
            src = ctxT_sb if jh == 0 else xT_sb
            pss = [pSim.tile([128, 512], f32, tag="sim",
                             name=f"kvps{2 * jh + half}") for half in range(2)]
            for kt in range(8):
                for half in range(2):
                    nc.tensor.matmul(
                        pss[half][:],
                        lhsT=wkv_sb[:, kt, :],
                        rhs=src[:, kt, half * 512:(half + 1) * 512],
                        start=(kt == 0), stop=(kt == 7))
            for half in range(2):
                c = 2 * jh + half
                js = slice(jh * 1024 + half * 512, jh * 1024 + half * 512 + 512)
                nc.vector.tensor_copy(out=kvT_sb[:, js], in_=pss[half][:])
                sqk = sqp.tile([64, 512], bf16, tag="sqk", name=f"sqk{c}")
                nc.scalar.activation(sqk[:], pss[half][0:64, :], Square)
                nc.tensor.matmul(
                    kssq[0:4, :], lhsT=indk_sb[:, c, :], rhs=sqk[:],
                    start=(c == 0), stop=(c == 3))

        emit_kv(0)
        emit_kv(1)

        # v transposes to j-major, build v_aug
        nc.vector.memset(va_sb[:, :, DH:DH + 1], 1.0)
        vt = pAv.tile([128, 1024], bf16, tag="av", name="vt")
        for jt in range(16):
            nc.tensor.transpose(
                vt[:, jt * 64:(jt + 1) * 64],
                kvT_sb[64:128, jt * 128:(jt + 1) * 128],
                idup_sb[64:128, :])
        nc.vector.tensor_copy(out=va_sb[:, :, 0:DH],
                              in_=vt[:].rearrange("p (t d) -> p t d", d=64))

        # ---- k normalization ----
        kln_sb = nrm.tile([16, 512], f32, tag="kln")
        nc.scalar.activation(kln_sb[0:4, :], kssq[0:4, :], Ln, bias=eps_sb[0:4])
        rsqK_sb = nrm.tile([16, 512], bf16, tag="rsqK")
        nc.scalar.activation(rsqK_sb[0:4, :], kln_sb[0:4, :], Exp, scale=-0.5)
        for c in range(4):
            js = slice(c * 512, (c + 1) * 512)
            kbc = pAux.tile([128, 512], f32, tag="aux", name=f"kbc{c}")
            nc.tensor.matmul(kbc[0:64, :], lhsT=selk_sb[:, c, :],
                             rhs=rsqK_sb[0:4, :], start=True, stop=True)
            nc.vector.tensor_mul(kn_sb[0:64, js], kvT_sb[0:64, js],
                                 kbc[0:64, :])
        nc.vector.tensor_scalar_mul(kn_sb[0:64, :], kn_sb[0:64, :], sdk_sb[:])
        # duplicate k into partitions 64-127 (odd heads' PE row group)
        nc.gpsimd.dma_start(out=kn_sb[64:128, :], in_=kn_sb[0:64, :])

        # ---- q projection + normalization, two half-batches (ft 0-1, 2-3)
        # so the band matmuls for early head-pairs fill the norm latency ----
        def emit_q_batch(batch):
            fts = (0, 1) if batch == 0 else (2, 3)
            qssq = pAux.tile([128, 512], f32, tag="aux", name=f"qssq{batch}")
            for ft in fts:
                # two psum banks interleaved over the halves
                pss = [pSim.tile([128, 512], f32, tag="sim",
                                 name=f"qps{2 * ft + half}")
                       for half in range(2)]
                for kt in range(8):
                    for half in range(2):
                        nc.tensor.matmul(
                            pss[half][:],
                            lhsT=wc_sb[:, kt, ft * 128:(ft + 1) * 128],
                            rhs=xT_sb[:, kt, half * 512:(half + 1) * 512],
                            start=(kt == 0), stop=(kt == 7))
                for half in range(2):
                    c = 2 * ft + half
                    qs = slice(half * 512, (half + 1) * 512)
                    nc.vector.tensor_copy(out=qn_sb[:, ft, qs],
                                          in_=pss[half][:])
                    sqq = sqp.tile([128, 512], bf16, tag="sqq",
                                   name=f"sqq{c}")
                    nc.scalar.activation(sqq[:], pss[half][:], Square)
                    nc.tensor.matmul(
                        qssq[0:8, :], lhsT=indq_sb[:, c, :], rhs=sqq[:],
                        start=(c % 4 == 0), stop=(c % 4 == 3))
            qln_sb = nrm.tile([16, 512], f32, tag=f"qln{batch}")
            nc.scalar.activation(qln_sb[0:8, :], qssq[0:8, :], Ln,
                                 bias=eps_sb[0:8])
            rsqQ_sb = nrm.tile([16, 512], bf16, tag=f"rsqQ{batch}")
            nc.scalar.activation(rsqQ_sb[0:8, :], qln_sb[0:8, :], Exp,
                                 scale=-0.5)
            for ft in fts:
                for half in range(2):
                    c = 2 * ft + half
                    qs = slice(half * 512, (half + 1) * 512)
                    qbc = pAux.tile([128, 512], f32, tag="aux",
                                    name=f"qbc{c}")
                    nc.tensor.matmul(qbc[:], lhsT=selq_sb[:, c, :],
                                     rhs=rsqQ_sb[0:8, :], start=True,
                                     stop=True)
                    nc.vector.tensor_mul(qn_sb[:, ft, qs], qn_sb[:, ft, qs],
                                         qbc[:])

        # bias prefetch machinery (DMAs on the sync queue, 2 pairs deep)
        bt_cache = {}

        def ensure_bt(h, qc):
            if (h, qc) in bt_cache:
                return bt_cache[(h, qc)]
            wtot = len(PACK[qc]) * 512
            bt = btp.tile([128, PTW], bf16, tag="bt", name=f"bt{h}_{qc}")
            nc.sync.dma_start(out=bt[:, 0:wtot],
                              in_=biasP[h * 2 + qc, :, 0:wtot])
            bt_cache[(h, qc)] = bt
            return bt

        ensure_bt(0, 0)
        ensure_bt(1, 0)

        # ---- band over prefix cols; additive window mask via identity-MM ----
        ptbs = []
        for h in range(HL):
            ptbs.append(ptbp.tile([128, 8, BW], bf16, tag="ptb",
                                  name=f"ptb{h}"))
        # q half-batches interleaved with band pairs: the band matmuls fill
        # the PE while each q half-batch's norm chain completes
        emit_q_batch(0)

        def emit_band(hp):
            for sub in range(4):          # 2 cts per 512-wide psum tile
                ct0 = 2 * sub
                bpss = [pSim.tile([128, 512], f32, tag="sim",
                                  name=f"bps{hp}_{sub}_{k}") for k in range(2)]
                for i in range(2):
                    ct = ct0 + i
                    c0 = 128 * ct
                    qw = min(BW, N - c0)
                    for pr in range(2):
                        base = pr * 64
                        nc.tensor.matmul(
                            bpss[pr][:, i * 256:i * 256 + qw],
                            lhsT=kn_sb[base:base + 64, c0:c0 + 128],
                            rhs=qn_sb[base:base + 64, hp, c0:c0 + qw],
                            start=(i == 0), stop=False)
                for pr in range(2):
                    nc.tensor.matmul(
                        bpss[pr][:], lhsT=i128_sb, rhs=lband2_sb,
                        start=False, stop=True)
                for pr in range(2):
                    ptb = ptbs[2 * hp + pr]
                    bview = bpss[pr][:].rearrange(
                        "p (i x) -> p i x", x=256)[:, :, 0:BW]
                    nc.scalar.activation(ptb[:, ct0:ct0 + 2, :], bview, Exp)

        emit_band(0)
        emit_band(1)
        emit_q_batch(1)
        emit_band(2)
        emit_band(3)

        # ---- per query-chunk: packed sims + bias accumulate -> exp -> AV ->
        # denominators (collected via PE) -> broadcast -> att ----
        def emit_sims_pair(hp, qc):
            Q0 = qc * 512
            tiles = PACK[qc]
            bts = [ensure_bt(2 * hp + pr, qc) for pr in range(2)]
            # prefetch the next pair's bias while this pair computes
            nh = 2 * hp + 2
            if nh < HL:
                for prn in range(2):
                    ensure_bt(nh + prn, qc)
            ptxs = []
            for pr in range(2):
                h = 2 * hp + pr
                ptxs.append(ptxp.tile([128, PTW], bf16, tag="ptx",
                                      name=f"ptx{h}_{qc}"))
            for tidx, segs in enumerate(tiles):
                used = USED[qc][tidx]
                sps2 = [pSim.tile([128, 512], f32, tag="sim",
                                  name=f"sps{hp}_{qc}_{tidx}_{k}")
                        for k in range(2)]
                for si, (ct, poff, offq, w) in enumerate(segs):
                    c0 = 128 * ct
                    for pr in range(2):
                        base = pr * 64
                        nc.tensor.matmul(
                            sps2[pr][:, poff:poff + w],
                            lhsT=kn_sb[base:base + 64, P + c0:P + c0 + 128],
                            rhs=qn_sb[base:base + 64, hp,
                                      Q0 + offq:Q0 + 512],
                            start=(si == 0), stop=False)
                for pr in range(2):
                    nc.tensor.matmul(
                        sps2[pr][:, 0:used], lhsT=i128_sb,
                        rhs=bts[pr][:, tidx * 512:tidx * 512 + used],
                        start=False, stop=True)
                for pr in range(2):
                    reg = slice(tidx * 512, tidx * 512 + used)
                    nc.scalar.activation(ptxs[pr][:, reg], sps2[pr][:, 0:used],
                                         Exp)
            return ptxs

        def emit_av_pair(hp, qc, ptxs, denC, avcs):
            Q0 = qc * 512
            tiles = PACK[qc]
            lists = []
            apss = []
            for pr in range(2):
                h = 2 * hp + pr
                ptx = ptxs[pr]
                aps = pAv.tile([128, 512], f32, tag="av", name=f"av{h}_{qc}")
                apss.append(aps)
                av_mms = []
                for tidx, segs in enumerate(tiles):
                    for (ct, poff, offq, w) in segs:
                        av_mms.append((
                            aps[0:DH + 1, offq:512], va_sb[:, 8 + ct, :],
                            ptx[:, tidx * 512 + poff:tidx * 512 + poff + w]))
                ptb = ptbs[h]
                for ct in range(8):
                    c0 = 128 * ct
                    qw = min(BW, N - c0)
                    lo = max(c0, Q0)
                    hi = min(c0 + qw, Q0 + 512)
                    if lo >= hi:
                        continue
                    av_mms.append((
                        aps[0:DH + 1, lo - Q0:hi - Q0], va_sb[:, ct, :],
                        ptb[:, ct, lo - c0:hi - c0]))
                lists.append(av_mms)
            # interleave the two heads' accumulate chains across psum banks
            nmax = max(len(x) for x in lists)
            for i in range(nmax):
                for pr in range(2):
                    if i < len(lists[pr]):
                        o, l, r = lists[pr][i]
                        nc.tensor.matmul(o, lhsT=l, rhs=r, start=(i == 0),
                                         stop=(i == len(lists[pr]) - 1))
            for pr in range(2):
                h = 2 * hp + pr
                aps = apss[pr]
                # stash av+denominator (bf16) and collect the denom row into
                # row h of the shared psum tile via a tiny contract-1 matmul
                avc = avcs[h]
                nc.vector.tensor_copy(out=avc[0:DH + 1, :],
                                      in_=aps[0:DH + 1, :])
                nc.tensor.matmul(denC[0:8, :],
                                 lhsT=selh1_sb[DH:DH + 1, h, :],
                                 rhs=avc[DH:DH + 1, :],
                                 start=(h == 0), stop=(h == HL - 1))

        def begin_qc(qc):
            denC = pAux.tile([128, 512], f32, tag="aux", name=f"denC{qc}")
            avcs = [recp.tile([128, 512], bf16, tag=f"avc{h}",
                              name=f"avc{h}_{qc}") for h in range(HL)]
            return denC, avcs

        def finish_qc(qc, denC, avcs):
            Q0 = qc * 512
            # batched 1/denom: ln+exp(-x), then select-matmul broadcast and
            # the att normalize multiplies
            dln = recp.tile([16, 512], f32, tag="dln", name=f"dln{qc}")
            nc.scalar.activation(dln[0:8, :], denC[0:8, :], Ln)
            rec_sb = recp.tile([16, 512], bf16, tag="rec", name=f"rec{qc}")
            nc.scalar.activation(rec_sb[0:8, :], dln[0:8, :], Exp, scale=-1.0)
            for h in range(HL):
                hp, pr = h // 2, h % 2
                # alternate denb between psum pools for 2-deep pipelining
                pool = pAux if h % 2 == 0 else pAv
                tag = "aux" if h % 2 == 0 else "av"
                denb = pool.tile([128, 512], f32, tag=tag,
                                 name=f"denb{h}_{qc}")
                nc.tensor.matmul(denb[0:64, :], lhsT=selh_sb[:, h, :],
                                 rhs=rec_sb[0:8, :], start=True, stop=True)
                base = pr * 64
                nc.vector.tensor_mul(
                    att_sb[base:base + 64, hp, Q0:Q0 + 512],
                    avcs[h][0:64, :], denb[0:64, :])
            # out-proj for this chunk, two et-chains interleaved across banks
            for ep in range(4):
                opss = [pAv.tile([128, 512], f32, tag="av",
                                 name=f"op{qc}_{2 * ep + k}")
                        for k in range(2)]
                for ftile in range(4):
                    for k in range(2):
                        et = 2 * ep + k
                        nc.tensor.matmul(
                            opss[k][:],
                            lhsT=wo_sb[:, ftile, et * 128:(et + 1) * 128],
                            rhs=att_sb[:, ftile, qc * 512:(qc + 1) * 512],
                            start=(ftile == 0), stop=(ftile == 3))
                for k in range(2):
                    et = 2 * ep + k
                    o_sb = osbp.tile([128, 512], bf16, tag="osb",
                                     name=f"osb{qc}_{et}")
                    if k == 0:
                        nc.vector.tensor_copy(out=o_sb[:], in_=opss[k][:])
                    else:
                        nc.scalar.copy(out=o_sb[:], in_=opss[k][:])
                    eng = nc.sync if k == 0 else nc.gpsimd
                    eng.dma_start(
                        out=outT[et * 128:(et + 1) * 128,
                                 qc * 512:(qc + 1) * 512],
                        in_=o_sb[:])

        # software-pipelined schedule: each pair's AV is emitted only after
        # the NEXT pair's sim matmuls (so the in-order PE queue never waits
        # on an Exp), and qc1's first pair fills the qc0 epilogue
        st0 = begin_qc(0)
        st1 = None
        px = {}
        px[(0, 0)] = emit_sims_pair(0, 0)
        for hp in range(1, 4):
            px[(hp, 0)] = emit_sims_pair(hp, 0)
            emit_av_pair(hp - 1, 0, px.pop((hp - 1, 0)), *st0)
        st1 = begin_qc(1)
        px[(0, 1)] = emit_sims_pair(0, 1)
        emit_av_pair(3, 0, px.pop((3, 0)), *st0)
        finish_qc(0, *st0)
        for hp in range(1, 4):
            px[(hp, 1)] = emit_sims_pair(hp, 1)
            emit_av_pair(hp - 1, 1, px.pop((hp - 1, 1)), *st1)
        emit_av_pair(3, 1, px.pop((3, 1)), *st1)
        finish_qc(1, *st1)

    return nc


_NC = None


def _get_nc():
    global _NC
    if _NC is None:
        _NC = _build_nc()
    return _NC


def _to_kt(mT):
    """[DIM, W] -> [128, 8, W] bf16, contract d = kt*128 + p."""
    import ml_dtypes
    bf = ml_dtypes.bfloat16
    W = mT.shape[1]
    return np.ascontiguousarray(
        mT.reshape(8, 128, W).transpose(1, 0, 2)).astype(bf)


def _prep_in_maps(x, prefix_context, attn_bias, gamma, Wq, Wkv, q_scale,
                  k_scale, Wo, mask):
    import ml_dtypes
    bf = ml_dtypes.bfloat16

    x = np.asarray(x, np.float32)
    prefix_context = np.asarray(prefix_context, np.float32)
    attn_bias = np.asarray(attn_bias, np.float32)
    gamma = np.asarray(gamma, np.float32)
    Wq = np.asarray(Wq, np.float32)
    Wkv = np.asarray(Wkv, np.float32)
    q_scale = np.asarray(q_scale, np.float32)
    k_scale = np.asarray(k_scale, np.float32)
    Wo = np.asarray(Wo, np.float32)
    mask = np.asarray(mask)

    killu = np.tril(np.ones((N, N), bool), -1)  # key c > query i -> masked
    sdk_np = (8.0 * q_scale * k_scale).astype(np.float32).reshape(DH, 1)
    wkv_kt = _to_kt(np.ascontiguousarray(Wkv.T))

    in_maps = []
    for c in CORES:
        b, g = c // 2, c % 2
        hs = slice(g * HL, (g + 1) * HL)
        # additive ln-domain bias [h, key, query] with causal/key-mask kills
        lb = attn_bias[hs].transpose(0, 2, 1).copy()
        lb[:, killu] = NEGB
        maskf = mask[b]
        if not maskf.all():
            lb[:, ~maskf, :] = NEGB
        # pack into the on-chip psum column layout: [h*2+qc, 128, PTW]
        lbp = np.zeros((HL * 2, 128, PTW), np.float32)
        for h in range(HL):
            for qc in range(2):
                Q0 = qc * 512
                for tidx, segs in enumerate(PACK[qc]):
                    for (ct, poff, offq, w) in segs:
                        lbp[h * 2 + qc, :, tidx * 512 + poff:
                            tidx * 512 + poff + w] = \
                            lb[h, 128 * ct:128 * (ct + 1),
                               Q0 + offq:Q0 + offq + w]
        Wg = Wq[g * FL:(g + 1) * FL] * gamma[None, :]
        s = Wg.sum(axis=1)
        wcT = Wg.T - s[None, :] / DIM
        wog = Wo[:, g * FL:(g + 1) * FL].T              # [FL, DIM]
        wo_kt = np.ascontiguousarray(
            wog.reshape(4, 128, DIM).transpose(1, 0, 2)).astype(bf)
        in_maps.append(dict(
            xT=_to_kt(np.ascontiguousarray(x[b].T)),
            ctxT=_to_kt(np.ascontiguousarray(prefix_context[b].T)),
            biasP=np.ascontiguousarray(lbp).astype(bf),
            wc=_to_kt(wcT),
            wkv=wkv_kt,
            wo=wo_kt,
            sdk=sdk_np,
        ))
    return in_maps


def kernel(**inputs):
    from concourse.bass_utils import run_bass_kernel_spmd

    nc = _get_nc()
    in_maps = _prep_in_maps(**inputs)
    res = run_bass_kernel_spmd(nc, in_maps, CORES).results
    out = np.empty((B, N, DIM), np.float32)
    for b in range(B):
        out[b] = (np.asarray(res[2 * b]["outT"], np.float32)
                  + np.asarray(res[2 * b + 1]["outT"], np.float32)).T
    return out


# revision 53
# speedup vs baseline: 1.0119x; 1.0119x over previous
# Distributed sparse-attention kernel for Trainium2 (8 NeuronCores).
#
# Sharding: core c = (batch b = c//2, head-group g = c%2 of 8 heads).
# Each core computes, for its (b, g):
#   q  = meancenter(x) @ Wc          (LN rstd cancels under l2norm; gamma and
#                                     mean-centering folded into Wc on host)
#   kv = [prefix; x] @ Wkv.T         (MQA single head, replicated per batch)
#   qn = l2norm(q) ; kn = l2norm(k) * (8 * q_scale * k_scale)
#   logits[c_key, i_query] = kn.T qn  (transposed; causal triangle over x-cols
#                                     packed edge-to-edge in psum, 16-wide band
#                                     over prefix-cols)
#   logits += bias                   (additive ln-domain bias incl. causal /
#                                     key-mask kills, accumulated into psum by
#                                     identity matmuls on the PE)
#   P = exp(logits)                  (single activation, final P)
#   avT[d, i] = sum_j P[j, i] v[j, d], denom via an appended ones column of v
#   outT_partial = Wo_g.T @ (avT / denom)
# Host sums the two head-group partials per batch and transposes back.

import numpy as np

B, N, P, DIM, HEADS, DH = 4, 1024, 1024, 1024, 16, 64
HL = 8                 # heads per core
FL = HL * DH           # 512 local q features
J = P + N              # 2048 keys
WIND = 16              # prefix cond-window
BW = 144               # band tile width (128 cols + 16 window - 1, padded)
NEGB = -30000.0        # masked-logit bias (finite: 0 * -inf = NaN in matmuls)
CORES = list(range(8))

# packed psum column layout for the causal sims, per query-chunk qc:
# list of single-bank (512-wide) tiles; each tile is a list of
# (ct, pack_off, off_q, width); the Exp covers only valid columns.
PACK = {
    0: [[(0, 0, 0, 512)],
        [(1, 0, 128, 384), (3, 384, 384, 128)],
        [(2, 0, 256, 256)]],
    1: [[(0, 0, 0, 512)], [(1, 0, 0, 512)], [(2, 0, 0, 512)],
        [(3, 0, 0, 512)], [(4, 0, 0, 512)],
        [(5, 0, 128, 384), (7, 384, 384, 128)],
        [(6, 0, 256, 256)]],
}
USED = {0: [512, 512, 256], 1: [512] * 6 + [256]}
PTW = 3584             # per-(head, qc) packed width: max tiles (7) * 512


def _build_consts():
    """One packed [128, W] bf16 constant tensor; returns (array, offsets)."""
    cols = {}
    parts = []
    w = 0

    def add(name, arr):
        nonlocal w
        a = np.zeros((128, arr.shape[-1] if arr.ndim == 2 else
                      arr.shape[1] * arr.shape[2]), np.float32)
        if arr.ndim == 2:
            a[:arr.shape[0], :] = arr
        else:
            a[:arr.shape[0], :] = arr.reshape(arr.shape[0], -1)
        cols[name] = w
        parts.append(a)
        w += a.shape[1]

    # idup: v-transpose identity  [128, 64]
    add("idup", (np.arange(128)[:, None] % 64 == np.arange(64)[None, :])
        .astype(np.float32))
    # i128: full identity for psum bias-accumulate matmuls
    add("i128", np.eye(128, dtype=np.float32))
    # lband2: additive band mask, two 256-strided window copies [128, 512]
    lb = np.full((128, 512), NEGB, np.float32)
    r = np.arange(128)[:, None]
    t = np.arange(256)[None, :]
    win = ((t - r >= 0) & (t - r < WIND))
    lb[:, 0:256][win] = 0.0
    lb[:, 256:512][win] = 0.0
    add("lband2", lb)
    # indk[d, c, r] = (r == c): k ssq accumulate rows  [64, 4, 4]
    indk = np.zeros((64, 4, 4), np.float32)
    for c in range(4):
        indk[:, c, c] = 1.0
    add("indk", indk)
    # indq[d, c, r]: q ssq chunk c -> rows 2*(c%4), 2*(c%4)+1 of its half-batch
    indq = np.zeros((128, 8, 8), np.float32)
    for c in range(8):
        indq[0:64, c, 2 * (c % 4)] = 1.0
        indq[64:128, c, 2 * (c % 4) + 1] = 1.0
    add("indq", indq)
    # selk[k, c, p]: broadcast rsqK row c -> 64 partitions  [4, 4, 64]
    selk = np.zeros((4, 4, 64), np.float32)
    for c in range(4):
        selk[c, c, :] = 1.0
    add("selk", selk)
    # selq[k, c, p]: broadcast rsqQ half-batch rows  [8, 8, 128]
    selq = np.zeros((8, 8, 128), np.float32)
    for c in range(8):
        selq[2 * (c % 4), c, 0:64] = 1.0
        selq[2 * (c % 4) + 1, c, 64:128] = 1.0
    add("selq", selq)
    # selh[k, h, p]: broadcast denom-reciprocal row h  [8, 8, 64]
    selh = np.zeros((8, 8, 64), np.float32)
    for h in range(8):
        selh[h, h, :] = 1.0
    add("selh", selh)
    # selh1[p, h, j]: collect denom row -> psum row h (contract 1)  [128, 8, 8]
    selh1 = np.zeros((128, 8, 8), np.float32)
    for h in range(8):
        selh1[:, h, h] = 1.0
    add("selh1", selh1)

    return np.concatenate(parts, axis=1), cols


def _patch_tile_drain():
    """walrus in this image only encodes ~2 sem waits on a CTRL (Drain/Nop)
    instruction; Tile's exit drain attaches every outstanding sem wait to a
    single drain.  Split the waits across extra sync-engine nops."""
    import concourse.tile as tile_mod
    from concourse import mybir
    from concourse.vector_clock import ScopedClock

    if getattr(tile_mod.TileContext, "_drain_split_patch", False):
        return
    MAXW = 1

    _ENGS = {
        mybir.EngineType.PE, mybir.EngineType.Activation,
        mybir.EngineType.Pool, mybir.EngineType.DVE, mybir.EngineType.SP,
    }
    _LIMITS = {}
    _nsplit = [0]
    orig_add = tile_mod.TileContext._add_instruction

    def _add_instruction(self, inst):
        si = inst.sync_info
        lim = _LIMITS.get(inst.engine, 1)
        if (si is not None and si.on_wait and len(si.on_wait) > lim
                and inst.engine in _ENGS):
            waits = list(si.on_wait)
            keep = waits[:lim]
            rest = waits[lim:]
            inst.sync_info = mybir.SyncInfo(
                on_wait=keep, on_update=list(si.on_update or []))
            for i in range(0, len(rest), MAXW):
                _nsplit[0] += 1
                nop = mybir.InstNoOp(
                    name=f"{inst.name}-ws{_nsplit[0]}", ins=[], outs=[])
                nop.engine = inst.engine
                nop.sync_info = mybir.SyncInfo(
                    on_wait=rest[i:i + MAXW], on_update=[])
                orig_add(self, nop)
        orig_add(self, inst)

    tile_mod.TileContext._add_instruction = _add_instruction

    def _drain_and_barrier(self, tick_clock, wait_clock):
        drain_inst = self.nc.sync.drain()
        wait_clock.add_sem_waits(
            drain_inst.ins, ScopedClock({None: tick_clock.global_clock})
        )
        si = drain_inst.ins.sync_info
        waits = list(si.on_wait or []) if si is not None else []
        if len(waits) > MAXW:
            ups = list(si.on_update or []) if si is not None else []
            drain_inst.ins.sync_info = mybir.SyncInfo(on_wait=[], on_update=ups)
            for i in range(0, len(waits), MAXW):
                nop = self.nc.sync.nop(nofuse=True)
                nop.ins.sync_info = mybir.SyncInfo(
                    on_wait=waits[i:i + MAXW], on_update=[])
        self.nc.all_engine_barrier()
        assert self.sems is not None
        popped = self.nc._tile_sem_poison_stack.pop()
        assert popped is self._sem_poison
        self.nc.clear_and_free_semaphores(list(self.sems.allocated().values()))
        self.nc.all_engine_barrier()

    tile_mod.TileContext._drain_and_barrier = _drain_and_barrier
    tile_mod.TileContext._drain_split_patch = True


def _build_nc():
    import ml_dtypes
    import concourse.bass as bass
    import concourse.tile as tile
    from concourse import mybir

    _patch_tile_drain()

    f32 = mybir.dt.float32
    bf16 = mybir.dt.bfloat16
    bf = ml_dtypes.bfloat16

    nc = bass.Bass("TRN2", target_bir_lowering=False, debug=False)

    xT = nc.dram_tensor("xT", [128, 8, N], bf16, kind="ExternalInput").ap()
    ctxT = nc.dram_tensor("ctxT", [128, 8, P], bf16,
                          kind="ExternalInput").ap()
    wc = nc.dram_tensor("wc", [128, 8, FL], bf16, kind="ExternalInput").ap()
    wkv = nc.dram_tensor("wkv", [128, 8, 2 * DH], bf16,
                         kind="ExternalInput").ap()
    wo = nc.dram_tensor("wo", [128, 4, DIM], bf16, kind="ExternalInput").ap()
    # additive ln-domain bias, packed to the psum column layout
    biasP = nc.dram_tensor("biasP", [HL * 2, 128, PTW], bf16,
                           kind="ExternalInput").ap()
    sdk = nc.dram_tensor("sdk", [DH, 1], f32, kind="ExternalInput").ap()
    outT = nc.dram_tensor("outT", [DIM, N], bf16, kind="ExternalOutput").ap()

    cst_np, CO = _build_consts()
    cst_dram = nc.inline_tensor(cst_np.astype(bf), "cstp").ap()

    Exp = mybir.ActivationFunctionType.Exp
    Ln = mybir.ActivationFunctionType.Ln

    with tile.TileContext(nc) as tc, \
            tc.tile_pool(name="big", bufs=1) as big, \
            tc.tile_pool(name="cst", bufs=1) as cst, \
            tc.tile_pool(name="sq", bufs=2) as sqp, \
            tc.tile_pool(name="nrm", bufs=1) as nrm, \
            tc.tile_pool(name="ptx", bufs=4) as ptxp, \
            tc.tile_pool(name="btp", bufs=5) as btp, \
            tc.tile_pool(name="ptb", bufs=8) as ptbp, \
            tc.tile_pool(name="rec", bufs=2) as recp, \
            tc.tile_pool(name="osb", bufs=3) as osbp, \
            tc.tile_pool(name="pSim", bufs=4, space="PSUM") as pSim, \
            tc.tile_pool(name="pAv", bufs=2, space="PSUM") as pAv, \
            tc.tile_pool(name="pAux", bufs=2, space="PSUM") as pAux:

        # ---- phase A: finely-chunked loads so the kv/q matmuls trickle-start
        # as data lands; kv-path first ----
        ctxT_sb = big.tile([128, 8, P], bf16, tag="ctxT")
        nc.sync.dma_start(ctxT_sb[:, 0:1, :], ctxT[:, 0:1, :])
        nc.sync.dma_start(ctxT_sb[:, 1:2, :], ctxT[:, 1:2, :])
        for kt in range(2, 8, 2):
            nc.sync.dma_start(ctxT_sb[:, kt:kt + 2, :], ctxT[:, kt:kt + 2, :])
        wkv_sb = big.tile([128, 8, 2 * DH], bf16, tag="wkv")
        nc.gpsimd.dma_start(wkv_sb[:], wkv)
        cst_sb = cst.tile([128, cst_np.shape[1]], bf16, tag="cstp")
        nc.gpsimd.dma_start(cst_sb[:], cst_dram)
        wc_sb = big.tile([128, 8, FL], bf16, tag="wc")
        nc.gpsimd.dma_start(wc_sb[:], wc)
        xT_sb = big.tile([128, 8, N], bf16, tag="xT")
        for kt in range(0, 8, 2):
            eng = nc.gpsimd if kt < 4 else nc.sync
            eng.dma_start(xT_sb[:, kt:kt + 2, :], xT[:, kt:kt + 2, :])
        wo_sb = big.tile([128, 4, DIM], bf16, tag="wo")
        nc.gpsimd.dma_start(wo_sb[:], wo)
        sdk_sb = cst.tile([DH, 1], f32, tag="sdk")
        nc.gpsimd.dma_start(sdk_sb[:], sdk)
        eps_sb = cst.tile([128, 1], f32, tag="eps")
        nc.vector.memset(eps_sb[:], 1e-24)

        def cview(name, shape=None):
            o = CO[name]
            if shape is None:
                return cst_sb[:, o:o + 128]
            w = int(np.prod(shape[1:]))
            v = cst_sb[:, o:o + w]
            if len(shape) == 3:
                v = v.rearrange("p (a b) -> p a b", b=shape[2])
                if shape[0] < 128:
                    v = v[0:shape[0], :, :]
            elif shape[0] < 128:
                v = v[0:shape[0], :]
            return v

        idup_sb = cst_sb[:, CO["idup"]:CO["idup"] + 64]
        i128_sb = cview("i128")
        lband2_sb = cst_sb[:, CO["lband2"]:CO["lband2"] + 512]
        indk_sb = cview("indk", (64, 4, 4))
        indq_sb = cview("indq", (128, 8, 8))
        selk_sb = cview("selk", (4, 4, 64))
        selq_sb = cview("selq", (8, 8, 128))
        selh_sb = cview("selh", (8, 8, 64))
        selh1_sb = cview("selh1", (128, 8, 8))

        kvT_sb = big.tile([128, J], bf16, tag="kvT")      # [2d, j] raw kv
        kn_sb = big.tile([128, J], bf16, tag="kn")        # normalized k, dup'd
        va_sb = big.tile([128, 16, DH + 1], bf16, tag="va")  # v_aug, j-major
        qn_sb = big.tile([128, 4, N], bf16, tag="qn")     # normalized q
        att_sb = big.tile([128, 4, N], bf16, tag="att")   # avT/denom

        # ---- kv projection (fp8 DoubleRow) + k sumsq ----
        kssq = pAux.tile([128, 512], f32, tag="aux", name="kssq")

        Square = mybir.ActivationFunctionType.Square

        def emit_kv(jh):
            # two psum banks interleaved so accumulate fills/drains overlap
            src = ctxT_sb if jh == 0 else xT_sb
            pss = [pSim.tile([128, 512], f32, tag="sim",
                             name=f"kvps{2 * jh + half}") for half in range(2)]
            for kt in range(8):
                for half in range(2):
                    nc.tensor.matmul(
                        pss[half][:],
                        lhsT=wkv_sb[:, kt, :],
                        rhs=src[:, kt, half * 512:(half + 1) * 512],
                        start=(kt == 0), stop=(kt == 7))
            for half in range(2):
                c = 2 * jh + half
                js = slice(jh * 1024 + half * 512, jh * 1024 + half * 512 + 512)
                nc.vector.tensor_copy(out=kvT_sb[:, js], in_=pss[half][:])
                sqk = sqp.tile([64, 512], bf16, tag="sqk", name=f"sqk{c}")
                nc.scalar.activation(sqk[:], pss[half][0:64, :], Square)
                nc.tensor.matmul(
                    kssq[0:4, :], lhsT=indk_sb[:, c, :], rhs=sqk[:],
                    start=(c == 0), stop=(c == 3))

        emit_kv(0)
        emit_kv(1)

        # v transposes to j-major, build v_aug
        nc.vector.memset(va_sb[:, :, DH:DH + 1], 1.0)
        vt = pAv.tile([128, 1024], bf16, tag="av", name="vt")
        for jt in range(16):
            nc.tensor.transpose(
                vt[:, jt * 64:(jt + 1) * 64],
                kvT_sb[64:128, jt * 128:(jt + 1) * 128],
                idup_sb[64:128, :])
        nc.vector.tensor_copy(out=va_sb[:, :, 0:DH],
                              in_=vt[:].rearrange("p (t d) -> p t d", d=64))

        # ---- k normalization ----
        kln_sb = nrm.tile([16, 512], f32, tag="kln")
        nc.scalar.activation(kln_sb[0:4, :], kssq[0:4, :], Ln, bias=eps_sb[0:4])
        rsqK_sb = nrm.tile([16, 512], bf16, tag="rsqK")
        nc.scalar.activation(rsqK_sb[0:4, :], kln_sb[0:4, :], Exp, scale=-0.5)
        for c in range(4):
            js = slice(c * 512, (c + 1) * 512)
            kbc = pAux.tile([128, 512], f32, tag="aux", name=f"kbc{c}")
            nc.tensor.matmul(kbc[0:64, :], lhsT=selk_sb[:, c, :],
                             rhs=rsqK_sb[0:4, :], start=True, stop=True)
            nc.vector.tensor_mul(kn_sb[0:64, js], kvT_sb[0:64, js],
                                 kbc[0:64, :])
        nc.vector.tensor_scalar_mul(kn_sb[0:64, :], kn_sb[0:64, :], sdk_sb[:])
        # duplicate k into partitions 64-127 (odd heads' PE row group)
        nc.gpsimd.dma_start(out=kn_sb[64:128, :], in_=kn_sb[0:64, :])

        # ---- q projection + normalization, two half-batches (ft 0-1, 2-3)
        # so the band matmuls for early head-pairs fill the norm latency ----
        def emit_q_batch(batch):
            fts = (0, 1) if batch == 0 else (2, 3)
            qssq = pAux.tile([128, 512], f32, tag="aux", name=f"qssq{batch}")
            for ft in fts:
                # two psum banks interleaved over the halves
                pss = [pSim.tile([128, 512], f32, tag="sim",
                                 name=f"qps{2 * ft + half}")
                       for half in range(2)]
                for kt in range(8):
                    for half in range(2):
                        nc.tensor.matmul(
                            pss[half][:],
                            lhsT=wc_sb[:, kt, ft * 128:(ft + 1) * 128],
                            rhs=xT_sb[:, kt, half * 512:(half + 1) * 512],
                            start=(kt == 0), stop=(kt == 7))
                for half in range(2):
                    c = 2 * ft + half
                    qs = slice(half * 512, (half + 1) * 512)
                    nc.vector.tensor_copy(out=qn_sb[:, ft, qs],
                                          in_=pss[half][:])
                    sqq = sqp.tile([128, 512], bf16, tag="sqq",
                                   name=f"sqq{c}")
                    nc.scalar.activation(sqq[:], pss[half][:], Square)
                    nc.tensor.matmul(
                        qssq[0:8, :], lhsT=indq_sb[:, c, :], rhs=sqq[:],
                        start=(c % 4 == 0), stop=(c % 4 == 3))
            qln_sb = nrm.tile([16, 512], f32, tag=f"qln{batch}")
            nc.scalar.activation(qln_sb[0:8, :], qssq[0:8, :], Ln,
                                 bias=eps_sb[0:8])
            rsqQ_sb = nrm.tile([16, 512], bf16, tag=f"rsqQ{batch}")
            nc.scalar.activation(rsqQ_sb[0:8, :], qln_sb[0:8, :], Exp,
                                 scale=-0.5)
            for ft in fts:
                for half in range(2):
                    c = 2 * ft + half
                    qs = slice(half * 512, (half + 1) * 512)
                    qbc = pAux.tile([128, 512], f32, tag="aux",
                                    name=f"qbc{c}")
                    nc.tensor.matmul(qbc[:], lhsT=selq_sb[:, c, :],
                                     rhs=rsqQ_sb[0:8, :], start=True,
                                     stop=True)
                    nc.vector.tensor_mul(qn_sb[:, ft, qs], qn_sb[:, ft, qs],
                                         qbc[:])

        # bias prefetch machinery (DMAs on the sync queue, 2 pairs deep)
        bt_cache = {}

        def ensure_bt(h, qc):
            if (h, qc) in bt_cache:
                return bt_cache[(h, qc)]
            wtot = len(PACK[qc]) * 512
            bt = btp.tile([128, PTW], bf16, tag="bt", name=f"bt{h}_{qc}")
            nc.sync.dma_start(out=bt[:, 0:wtot],
                              in_=biasP[h * 2 + qc, :, 0:wtot])
            bt_cache[(h, qc)] = bt
            return bt

        ensure_bt(0, 0)
        ensure_bt(1, 0)

        # ---- band over prefix cols; additive window mask via identity-MM ----
        ptbs = []
        for h in range(HL):
            ptbs.append(ptbp.tile([128, 8, BW], bf16, tag="ptb",
                                  name=f"ptb{h}"))
        # q half-batches interleaved with band pairs: the band matmuls fill
        # the PE while each q half-batch's norm chain completes
        emit_q_batch(0)

        def emit_band(hp):
            for sub in range(4):          # 2 cts per 512-wide psum tile
                ct0 = 2 * sub
                bpss = [pSim.tile([128, 512], f32, tag="sim",
                                  name=f"bps{hp}_{sub}_{k}") for k in range(2)]
                for i in range(2):
                    ct = ct0 + i
                    c0 = 128 * ct
                    qw = min(BW, N - c0)
                    for pr in range(2):
                        base = pr * 64
                        nc.tensor.matmul(
                            bpss[pr][:, i * 256:i * 256 + qw],
                            lhsT=kn_sb[base:base + 64, c0:c0 + 128],
                            rhs=qn_sb[base:base + 64, hp, c0:c0 + qw],
                            start=(i == 0), stop=False)
                for pr in range(2):
                    nc.tensor.matmul(
                        bpss[pr][:], lhsT=i128_sb, rhs=lband2_sb,
                        start=False, stop=True)
                for pr in range(2):
                    ptb = ptbs[2 * hp + pr]
                    bview = bpss[pr][:].rearrange(
                        "p (i x) -> p i x", x=256)[:, :, 0:BW]
                    nc.scalar.activation(ptb[:, ct0:ct0 + 2, :], bview, Exp)

        emit_band(0)
        emit_band(1)
        emit_q_batch(1)
        emit_band(2)
        emit_band(3)

        # ---- per query-chunk: packed sims + bias accumulate -> exp -> AV ->
        # denominators (collected via PE) -> broadcast -> att ----
        def emit_sims_pair(hp, qc):
            Q0 = qc * 512
            tiles = PACK[qc]
            bts = [ensure_bt(2 * hp + pr, qc) for pr in range(2)]
            # prefetch bias ahead (rolling into the next qc); btp bufs=5 caps
            # live tiles at current pair (2) + 3 prefetched
            for dh in (2, 3, 4):
                t = 2 * hp + dh
                if t < HL:
                    ensure_bt(t, qc)
                elif qc == 0:
                    ensure_bt(t - HL, 1)
            ptxs = []
            for pr in range(2):
                h = 2 * hp + pr
                ptxs.append(ptxp.tile([128, PTW], bf16, tag="ptx",
                                      name=f"ptx{h}_{qc}"))
            for tidx, segs in enumerate(tiles):
                used = USED[qc][tidx]
                sps2 = [pSim.tile([128, 512], f32, tag="sim",
                                  name=f"sps{hp}_{qc}_{tidx}_{k}")
                        for k in range(2)]
                for si, (ct, poff, offq, w) in enumerate(segs):
                    c0 = 128 * ct
                    for pr in range(2):
                        base = pr * 64
                        nc.tensor.matmul(
                            sps2[pr][:, poff:poff + w],
                            lhsT=kn_sb[base:base + 64, P + c0:P + c0 + 128],
                            rhs=qn_sb[base:base + 64, hp,
                                      Q0 + offq:Q0 + 512],
                            start=(si == 0), stop=False)
                for pr in range(2):
                    nc.tensor.matmul(
                        sps2[pr][:, 0:used], lhsT=i128_sb,
                        rhs=bts[pr][:, tidx * 512:tidx * 512 + used],
                        start=False, stop=True)
                for pr in range(2):
                    reg = slice(tidx * 512, tidx * 512 + used)
                    nc.scalar.activation(ptxs[pr][:, reg], sps2[pr][:, 0:used],
                                         Exp)
            return ptxs

        def emit_av_pair(hp, qc, ptxs, denC, avcs):
            Q0 = qc * 512
            tiles = PACK[qc]
            lists = []
            apss = []
            for pr in range(2):
                h = 2 * hp + pr
                ptx = ptxs[pr]
                aps = pAv.tile([128, 512], f32, tag="av", name=f"av{h}_{qc}")
                apss.append(aps)
                av_mms = []
                for tidx, segs in enumerate(tiles):
                    for (ct, poff, offq, w) in segs:
                        av_mms.append((
                            aps[0:DH + 1, offq:512], va_sb[:, 8 + ct, :],
                            ptx[:, tidx * 512 + poff:tidx * 512 + poff + w]))
                ptb = ptbs[h]
                for ct in range(8):
                    c0 = 128 * ct
                    qw = min(BW, N - c0)
                    lo = max(c0, Q0)
                    hi = min(c0 + qw, Q0 + 512)
                    if lo >= hi:
                        continue
                    av_mms.append((
                        aps[0:DH + 1, lo - Q0:hi - Q0], va_sb[:, ct, :],
                        ptb[:, ct, lo - c0:hi - c0]))
                lists.append(av_mms)
            # interleave the two heads' accumulate chains across psum banks
            nmax = max(len(x) for x in lists)
            for i in range(nmax):
                for pr in range(2):
                    if i < len(lists[pr]):
                        o, l, r = lists[pr][i]
                        nc.tensor.matmul(o, lhsT=l, rhs=r, start=(i == 0),
                                         stop=(i == len(lists[pr]) - 1))
            for pr in range(2):
                h = 2 * hp + pr
                aps = apss[pr]
                # stash av+denominator (bf16) and collect the denom row into
                # row h of the shared psum tile via a tiny contract-1 matmul
                avc = avcs[h]
                nc.vector.tensor_copy(out=avc[0:DH + 1, :],
                                      in_=aps[0:DH + 1, :])
                nc.tensor.matmul(denC[0:8, :],
                                 lhsT=selh1_sb[DH:DH + 1, h, :],
                                 rhs=avc[DH:DH + 1, :],
                                 start=(h == 0), stop=(h == HL - 1))

        def begin_qc(qc):
            denC = pAux.tile([128, 512], f32, tag="aux", name=f"denC{qc}")
            avcs = [recp.tile([128, 512], bf16, tag=f"avc{h}",
                              name=f"avc{h}_{qc}") for h in range(HL)]
            return denC, avcs

        def finish_qc(qc, denC, avcs):
            Q0 = qc * 512
            # batched 1/denom: ln+exp(-x), then select-matmul broadcast and
            # the att normalize multiplies
            dln = recp.tile([16, 512], f32, tag="dln", name=f"dln{qc}")
            nc.scalar.activation(dln[0:8, :], denC[0:8, :], Ln)
            rec_sb = recp.tile([16, 512], bf16, tag="rec", name=f"rec{qc}")
            nc.scalar.activation(rec_sb[0:8, :], dln[0:8, :], Exp, scale=-1.0)
            for h in range(HL):
                hp, pr = h // 2, h % 2
                # alternate denb between psum pools for 2-deep pipelining
                pool = pAux if h % 2 == 0 else pAv
                tag = "aux" if h % 2 == 0 else "av"
                denb = pool.tile([128, 512], f32, tag=tag,
                                 name=f"denb{h}_{qc}")
                nc.tensor.matmul(denb[0:64, :], lhsT=selh_sb[:, h, :],
                                 rhs=rec_sb[0:8, :], start=True, stop=True)
                base = pr * 64
                nc.vector.tensor_mul(
                    att_sb[base:base + 64, hp, Q0:Q0 + 512],
                    avcs[h][0:64, :], denb[0:64, :])
            # out-proj for this chunk, two et-chains interleaved across banks
            for ep in range(4):
                opss = [pAv.tile([128, 512], f32, tag="av",
                                 name=f"op{qc}_{2 * ep + k}")
                        for k in range(2)]
                for ftile in range(4):
                    for k in range(2):
                        et = 2 * ep + k
                        nc.tensor.matmul(
                            opss[k][:],
                            lhsT=wo_sb[:, ftile, et * 128:(et + 1) * 128],
                            rhs=att_sb[:, ftile, qc * 512:(qc + 1) * 512],
                            start=(ftile == 0), stop=(ftile == 3))
                for k in range(2):
                    et = 2 * ep + k
                    o_sb = osbp.tile([128, 512], bf16, tag="osb",
                                     name=f"osb{qc}_{et}")
                    if k == 0:
                        nc.vector.tensor_copy(out=o_sb[:], in_=opss[k][:])
                    else:
                        nc.scalar.copy(out=o_sb[:], in_=opss[k][:])
                    eng = nc.sync if k == 0 else nc.gpsimd
                    eng.dma_start(
                        out=outT[et * 128:(et + 1) * 128,
                                 qc * 512:(qc + 1) * 512],
                        in_=o_sb[:])

        # software-pipelined schedule: each pair's AV is emitted only after
        # the NEXT pair's sim matmuls (so the in-order PE queue never waits
        # on an Exp), and qc1's first pair fills the qc0 epilogue
        st0 = begin_qc(0)
        st1 = None
        px = {}
        px[(0, 0)] = emit_sims_pair(0, 0)
        for hp in range(1, 4):
            px[(hp, 0)] = emit_sims_pair(hp, 0)
            emit_av_pair(hp - 1, 0, px.pop((hp - 1, 0)), *st0)
        st1 = begin_qc(1)
        px[(0, 1)] = emit_sims_pair(0, 1)
        emit_av_pair(3, 0, px.pop((3, 0)), *st0)
        px[(1, 1)] = emit_sims_pair(1, 1)
        emit_av_pair(0, 1, px.pop((0, 1)), *st1)
        finish_qc(0, *st0)
        for hp in range(2, 4):
            px[(hp, 1)] = emit_sims_pair(hp, 1)
            emit_av_pair(hp - 1, 1, px.pop((hp - 1, 1)), *st1)
        emit_av_pair(3, 1, px.pop((3, 1)), *st1)
        finish_qc(1, *st1)

    return nc


_NC = None


def _get_nc():
    global _NC
    if _NC is None:
        _NC = _build_nc()
    return _NC


def _to_kt(mT):
    """[DIM, W] -> [128, 8, W] bf16, contract d = kt*128 + p."""
    import ml_dtypes
    bf = ml_dtypes.bfloat16
    W = mT.shape[1]
    return np.ascontiguousarray(
        mT.reshape(8, 128, W).transpose(1, 0, 2)).astype(bf)


def _prep_in_maps(x, prefix_context, attn_bias, gamma, Wq, Wkv, q_scale,
                  k_scale, Wo, mask):
    import ml_dtypes
    bf = ml_dtypes.bfloat16

    x = np.asarray(x, np.float32)
    prefix_context = np.asarray(prefix_context, np.float32)
    attn_bias = np.asarray(attn_bias, np.float32)
    gamma = np.asarray(gamma, np.float32)
    Wq = np.asarray(Wq, np.float32)
    Wkv = np.asarray(Wkv, np.float32)
    q_scale = np.asarray(q_scale, np.float32)
    k_scale = np.asarray(k_scale, np.float32)
    Wo = np.asarray(Wo, np.float32)
    mask = np.asarray(mask)

    killu = np.tril(np.ones((N, N), bool), -1)  # key c > query i -> masked
    sdk_np = (8.0 * q_scale * k_scale).astype(np.float32).reshape(DH, 1)
    wkv_kt = _to_kt(np.ascontiguousarray(Wkv.T))

    in_maps = []
    for c in CORES:
        b, g = c // 2, c % 2
        hs = slice(g * HL, (g + 1) * HL)
        # additive ln-domain bias [h, key, query] with causal/key-mask kills
        lb = attn_bias[hs].transpose(0, 2, 1).copy()
        lb[:, killu] = NEGB
        maskf = mask[b]
        if not maskf.all():
            lb[:, ~maskf, :] = NEGB
        # pack into the on-chip psum column layout: [h*2+qc, 128, PTW]
        lbp = np.zeros((HL * 2, 128, PTW), np.float32)
        for h in range(HL):
            for qc in range(2):
                Q0 = qc * 512
                for tidx, segs in enumerate(PACK[qc]):
                    for (ct, poff, offq, w) in segs:
                        lbp[h * 2 + qc, :, tidx * 512 + poff:
                            tidx * 512 + poff + w] = \
                            lb[h, 128 * ct:128 * (ct + 1),
                               Q0 + offq:Q0 + offq + w]
        Wg = Wq[g * FL:(g + 1) * FL] * gamma[None, :]
        s = Wg.sum(axis=1)
        wcT = Wg.T - s[None, :] / DIM
        wog = Wo[:, g * FL:(g + 1) * FL].T              # [FL, DIM]
        wo_kt = np.ascontiguousarray(
            wog.reshape(4, 128, DIM).transpose(1, 0, 2)).astype(bf)
        in_maps.append(dict(
            xT=_to_kt(np.ascontiguousarray(x[b].T)),
            ctxT=_to_kt(np.ascontiguousarray(prefix_context[b].T)),
            biasP=np.ascontiguousarray(lbp).astype(bf),
            wc=_to_kt(wcT),
            wkv=wkv_kt,
            wo=wo_kt,
            sdk=sdk_np,
        ))
    return in_maps


def kernel(**inputs):
    from concourse.bass_utils import run_bass_kernel_spmd

    nc = _get_nc()
    in_maps = _prep_in_maps(**inputs)
    res = run_bass_kernel_spmd(nc, in_maps, CORES).results
    out = np.empty((B, N, DIM), np.float32)
    for b in range(B):
        out[b] = (np.asarray(res[2 * b]["outT"], np.float32)
                  + np.asarray(res[2 * b + 1]["outT"], np.float32)).T
    return out
